# revision 3
# baseline (speedup 1.0000x reference)
"""DimensionalConsistencyLoss on 8 Trainium2 NeuronCores.

The loss touches only gathered rows of the [100000, 512] f32 table: 8192
pos/neg constraints read one row each (sparsity term + target element), 2048
neu constraints read one element.

Vocab-sharded design (per the sharding hint): core c owns rows
[12500c, 12500(c+1)) and the ~1280 constraints whose word_id falls there.
Shard-local row ids fit int16, which unlocks the vectorized `dma_gather`
Q7 ucode (SIMD descriptor emission, ~16 descriptors per loop iteration)
instead of the scalar DMA_INDIRECT path (~9ns/descriptor serialized on the
GpSimd engine, ~1.15us per 128-row column).

Per core: CAP=1536 slots (12 columns of 128; pads use row 0 with all-zero
loss coefficients so they contribute 0):
  - 3x dma_gather of 512 rows each ([128, 4, 512] per call).
  - ACT: per column, activation(Abs, accum_out) -> rowsum in one pass.
  - DVE: per column, scalar_tensor_tensor((ramp == dim) * row, accum_out)
    extracts the target element t.
  - Per-slot coefficient columns (host-built; classes are mixed per column):
        L = a*(w*Q + Pp) + w*R + Cc*rowsum,  a = |t|, w = (t*S >= 0)
    pos: S=-1, Q=1.1, Pp=-0.1-C_SP, R=0.1, Cc=C_SP
    neg: S=+1, same Q/Pp/R/Cc;  neu: S=0 -> w=1, Q=R=Cc=0, Pp=2 -> L=2a
    pad: all coefficients 0 -> L=0.
  - out [128, 12] -> host sums 8 partial tiles and applies the final scale.

Only the 25.6MB shard of the table is uploaded per core (not the full
200MB), plus ~320KB of indices/coefficients.
"""

import numpy as np

import concourse.bacc as bacc
import concourse.bass as bass
import concourse.mybir as mybir
from concourse import library_config
from concourse.bass_utils import run_bass_kernel_spmd

P = 128
VOCAB = 100000
DIM = 512
N_POS = 4096
N_NEG = 4096
N_NEU = 2048
N_ALL = N_POS + N_NEG + N_NEU
N_CORES = 8
SHARD = VOCAB // N_CORES           # 12500 rows per core

CAP = 1536                         # max owned constraints per core (12.3 sigma)
COLS = CAP // P                    # 12
NGATH = 3                          # row gathers per core
GIDX = CAP // NGATH                # 512 indices per gather
GCOLS = COLS // NGATH              # 4 columns per gather

CONSISTENCY_WEIGHT = 0.5
SPARSITY_WEIGHT = 0.1
C_SP = SPARSITY_WEIGHT / (DIM - 1)

# coefs tensor layout (f32, [128, CW_TOT]): ramp | dims | S | Pp | Q | R | Cc
CW_RAMP = DIM
C_DIMS = CW_RAMP
C_S = C_DIMS + COLS
C_PP = C_S + COLS
C_Q = C_PP + COLS
C_R = C_Q + COLS
C_CC = C_R + COLS
CW_TOT = C_CC + COLS

F32 = mybir.dt.float32
I16 = mybir.dt.int16
OP = mybir.AluOpType
AF = mybir.ActivationFunctionType

_nc_cache = None


def _build_program():
    global _nc_cache
    if _nc_cache is not None:
        return _nc_cache

    nc = bacc.Bacc(
        "TRN2", target_bir_lowering=False, debug=False, num_devices=N_CORES,
    )
    emb = nc.dram_tensor("emb", [SHARD, DIM], F32, kind="ExternalInput")
    idx_d = nc.dram_tensor("idx16", [P, CAP // 16], I16, kind="ExternalInput")
    coef_d = nc.dram_tensor("coefs", [P, CW_TOT], F32, kind="ExternalInput")
    out_d = nc.dram_tensor("out", [P, COLS], F32, kind="ExternalOutput")

    from contextlib import ExitStack

    with ExitStack() as ctx:
        sb = lambda name, shape, dt=F32: ctx.enter_context(
            nc.sbuf_tensor(name, shape, dt)
        )
        idx_sb = sb("idx_sb", [P, CAP // 16], I16)
        coef_sb = sb("coef_sb", [P, CW_TOT])
        rows = sb("rows", [P, COLS, DIM])
        s_scr = sb("s_scr", [P, DIM])
        s_dve = sb("s_dve", [P, DIM])
        rowsum = sb("rowsum", [P, COLS])
        tcol = sb("tcol", [P, COLS])
        a12 = sb("a12", [P, COLS])
        u12 = sb("u12", [P, COLS])
        w12 = sb("w12", [P, COLS])
        x1 = sb("x1", [P, COLS])
        x2 = sb("x2", [P, COLS])
        x3 = sb("x3", [P, COLS])
        sem = lambda name: ctx.enter_context(nc.semaphore(name))
        io_i, io_c, io2 = sem("io_i"), sem("io_c"), sem("io2")
        gs = [sem(f"g{k}") for k in range(NGATH)]
        acts, dvx, dvf = sem("acts"), sem("dvx"), sem("dvf")
        ramp = coef_sb[:, 0:CW_RAMP]

        # Issue input loads before the Block so they overlap its entry.
        nc.sync.dma_start(idx_sb[:, :], idx_d[:, :]).then_inc(io_i, 16)
        nc.sync.dma_start(coef_sb[:, :], coef_d[:, :]).then_inc(io_c, 16)

        blk_ctx = nc.Block()
        block = blk_ctx.__enter__()

        @block.gpsimd
        def _(gpsimd: bass.BassGpSimd):
            # IRAM library load for dma_gather; skipped by the Q7 when the
            # mlp library is already resident. Issued before the idx wait so
            # a cold load overlaps the HWDGE input DMAs.
            gpsimd.load_library(library_config.mlp)
            gpsimd.wait_ge(io_i, 16)
            for k in range(NGATH):
                gpsimd.dma_gather(
                    rows[:, k * GCOLS : (k + 1) * GCOLS, :],
                    emb[:, :],
                    idx_sb[:, k * (GIDX // 16) : (k + 1) * (GIDX // 16)],
                    GIDX,
                    GIDX,
                    DIM,
                ).then_inc(gs[k], 16)

        @block.scalar
        def _(scalar: bass.BassEngine):
            for k in range(NGATH):
                scalar.wait_ge(gs[k], 16)
                for j in range(k * GCOLS, (k + 1) * GCOLS):
                    nc.scalar.activation(
                        s_scr[:, :], rows[:, j, :], AF.Abs,
                        accum_out=rowsum[:, j : j + 1],
                    ).then_inc(acts, 1)
            scalar.wait_ge(dvx, COLS)
            nc.scalar.activation(a12[:, :], tcol[:, :], AF.Abs).then_inc(acts, 1)

        @block.vector
        def _(vector: bass.BassEngine):
            vector.wait_ge(io_c, 16)
            for k in range(NGATH):
                vector.wait_ge(gs[k], 16)
                for j in range(k * GCOLS, (k + 1) * GCOLS):
                    nc.vector.scalar_tensor_tensor(
                        out=s_dve[:, :],
                        in0=ramp,
                        scalar=coef_sb[:, C_DIMS + j : C_DIMS + j + 1],
                        in1=rows[:, j, :],
                        op0=OP.is_equal,
                        op1=OP.mult,
                        accum_out=tcol[:, j : j + 1],
                    ).then_inc(dvx, 1)
            # accum_out writes land late; drain our own pipeline before reads
            vector.wait_ge(dvx, COLS)
            n = 0

            def step(ins, wait=None):
                nonlocal n
                if wait is not None:
                    vector.wait_ge(dvf, wait)
                ins().then_inc(dvf, 1)
                n += 1
                return n

            # L = a*(w*Q + Pp) + w*R + Cc*rowsum,  w = (t*S >= 0), a = |t|
            i_u = step(lambda: nc.vector.tensor_tensor(
                out=u12[:, :], in0=tcol[:, :],
                in1=coef_sb[:, C_S : C_S + COLS], op=OP.mult))
            i_w = step(lambda: nc.vector.tensor_scalar(
                out=w12[:, :], in0=u12[:, :], scalar1=0.0, scalar2=None,
                op0=OP.is_ge), wait=i_u)
            i1 = step(lambda: nc.vector.tensor_tensor(
                out=x1[:, :], in0=w12[:, :], in1=coef_sb[:, C_Q : C_Q + COLS],
                op=OP.mult), wait=i_w)
            i2 = step(lambda: nc.vector.tensor_tensor(
                out=x2[:, :], in0=w12[:, :], in1=coef_sb[:, C_R : C_R + COLS],
                op=OP.mult))
            i3 = step(lambda: nc.vector.tensor_tensor(
                out=x3[:, :], in0=rowsum[:, :],
                in1=coef_sb[:, C_CC : C_CC + COLS], op=OP.mult))
            i4 = step(lambda: nc.vector.tensor_tensor(
                out=x1[:, :], in0=x1[:, :], in1=coef_sb[:, C_PP : C_PP + COLS],
                op=OP.add), wait=i1)
            i5 = step(lambda: nc.vector.tensor_tensor(
                out=x2[:, :], in0=x2[:, :], in1=x3[:, :], op=OP.add),
                wait=max(i2, i3))
            vector.wait_ge(acts, COLS + 1)
            i6 = step(lambda: nc.vector.tensor_tensor(
                out=x1[:, :], in0=x1[:, :], in1=a12[:, :], op=OP.mult),
                wait=i4)
            i7 = step(lambda: nc.vector.tensor_tensor(
                out=x1[:, :], in0=x1[:, :], in1=x2[:, :], op=OP.add),
                wait=max(i6, i5))

        @block.sync
        def _(sync: bass.BassEngine):
            sync.wait_ge(dvf, 9)
            sync.dma_start(out_d[:, :], x1[:, :]).then_inc(io2, 16)
            sync.wait_ge(io2, 16)

        blk_ctx.__exit__(None, None, None)
        # The NEFF can be executed repeatedly on one load: clear our
        # semaphores after the end-of-block barrier so every run starts
        # from zero (same dance as Bass.reset()).
        ksr = nc._kernel_sem_range
        mono_start = ksr.start + 3 + (
            1 if nc._bir_kernel_barrier_sem is not None else 0
        )
        user_range = range(mono_start + len(nc._monotonic_sems), ksr.stop)
        nc.gpsimd.sem_clear(user_range)

    nc.compile()
    _nc_cache = nc
    return nc


def _deal(pos_ids, pos_dims, neg_ids, neg_dims, neu_ids, neu_dims):
    """Assign each constraint to the core owning its embedding row
    (core = id // SHARD). Returns per-core
    (idx16 [128, CAP//16] wrapped int16, coefs [128, CW_TOT] f32).

    idx wrapping (dma_gather contract): logical index n lives at
    wrapped[p, n // 16] for every p with p % 16 == n % 16 (replicated
    across the eight 16-partition groups).
    Output slot of logical index n: partition n % 128, column n // 128.
    """
    ids = np.concatenate([pos_ids, neg_ids, neu_ids]).astype(np.int64)
    dims = np.concatenate([pos_dims, neg_dims, neu_dims]).astype(np.int64)
    cls = np.concatenate([
        np.zeros(len(pos_ids), np.int64),
        np.ones(len(neg_ids), np.int64),
        np.full(len(neu_ids), 2, np.int64),
    ])

    idx16_maps = []
    coef_maps = []
    for c in range(N_CORES):
        m = (ids // SHARD) == c
        n_c = int(m.sum())
        assert n_c <= CAP, f"core {c} owns {n_c} > CAP={CAP} constraints"
        loc = np.zeros(CAP, np.int64)          # pad: row 0 of the shard
        loc[:n_c] = ids[m] - c * SHARD
        cdim = np.zeros(CAP, np.int64)
        cdim[:n_c] = dims[m]
        ccls = np.full(CAP, 3, np.int64)       # pad class
        ccls[:n_c] = cls[m]

        wrapped = np.ascontiguousarray(
            np.tile(loc.reshape(CAP // 16, 16).T, (N_CORES, 1)).astype(np.int16)
        )  # [128, CAP//16]

        cf = np.zeros((P, CW_TOT), np.float32)
        cf[:, 0:CW_RAMP] = np.arange(DIM, dtype=np.float32)[None, :]
        dm = cdim.reshape(COLS, P).T           # slot n -> (n%128, n//128)
        kl = ccls.reshape(COLS, P).T
        pn = kl <= 1
        cf[:, C_DIMS : C_DIMS + COLS] = dm
        cf[:, C_S : C_S + COLS] = np.where(kl == 0, -1.0, np.where(kl == 1, 1.0, 0.0))
        cf[:, C_PP : C_PP + COLS] = np.where(
            pn, -SPARSITY_WEIGHT - C_SP, np.where(kl == 2, 2.0, 0.0))
        cf[:, C_Q : C_Q + COLS] = np.where(pn, 1.0 + SPARSITY_WEIGHT, 0.0)
        cf[:, C_R : C_R + COLS] = np.where(pn, SPARSITY_WEIGHT, 0.0)
        cf[:, C_CC : C_CC + COLS] = np.where(pn, C_SP, 0.0)
        idx16_maps.append(wrapped)
        coef_maps.append(cf)
    return idx16_maps, coef_maps


def _make_in_maps(emb, pos_ids, pos_dims, neg_ids, neg_dims, neu_ids, neu_dims):
    idx16, coefs = _deal(pos_ids, pos_dims, neg_ids, neg_dims, neu_ids, neu_dims)
    return [
        {
            "emb": np.ascontiguousarray(emb[c * SHARD : (c + 1) * SHARD]),
            "idx16": idx16[c],
            "coefs": coefs[c],
        }
        for c in range(N_CORES)
    ]


def kernel(**inputs):
    emb = np.ascontiguousarray(np.asarray(inputs["embeddings"], dtype=np.float32))
    ids = {
        k: np.asarray(inputs[k]).astype(np.int64)
        for k in ("pos_ids", "pos_dims", "neg_ids", "neg_dims", "neu_ids", "neu_dims")
    }
    nc = _build_program()
    in_maps = _make_in_maps(
        emb, ids["pos_ids"], ids["pos_dims"], ids["neg_ids"], ids["neg_dims"],
        ids["neu_ids"], ids["neu_dims"],
    )
    res = run_bass_kernel_spmd(nc, in_maps, list(range(N_CORES)))
    total = sum(float(r["out"].astype(np.float64).sum()) for r in res.results)
    val = total * CONSISTENCY_WEIGHT / N_ALL
    return np.asarray(val, dtype=np.float32)


# revision 4
# speedup vs baseline: 1.5518x; 1.5518x over previous
"""DimensionalConsistencyLoss on 8 Trainium2 NeuronCores.

The loss touches only gathered rows of the [100000, 512] f32 table: 8192
pos/neg constraints read one row each (sparsity term + target element), 2048
neu constraints read one element. Everything is fetched with indirect DMA
row/element gathers (SWDGE emission is CPU-bound at ~9.3ns/descriptor, so
the 10-instruction gather window ~14.5us is the kernel's spine; everything
else is hidden under it).

Per core (1/8 of the constraints = 1280 slots = 10 columns of 128, dealt
round-robin by the host so every column is single-class: cols 0-3 pos,
4-7 neg, 8-9 neu):
  - idx table loaded by the GpSimd engine itself (SWDGE patterned DMA,
    CounterMachine emission) -- ready ~1us earlier than the HWDGE path.
  - 8x indirect-DMA row gathers (one [128,512] tile per column, striped
    over the 4 SWDGE queues), then 2x flat element gathers for neu
    (idx = id*512 + dim lands t directly); elements go LAST so the final
    instruction's drain is tiny.
  - ACT: per column, activation(Abs, accum_out) -> rowsum in one pass.
  - DVE: per column, scalar_tensor_tensor((ramp == dim) * row, accum_out)
    extracts t; the loss chain runs on [128,4] tiles with compile-time
    immediates, interleaved into the gather gaps. |t| is computed on the
    DVE as u*(2w-1) so the ACT engine is off the critical path entirely:
        pos: u = -t, neg/neu: u = t;  w = (u >= 0);  a = u*(2w-1) = |t|
        pos/neg: L = a*(1.1w - 0.1 - C_SP) + 0.1w + C_SP*rowsum
        neu:     L = 2a
  - out [128, 10] -> host sums 8 partial tiles and applies the final scale.
"""

import numpy as np

import concourse.bacc as bacc
import concourse.bass as bass
import concourse.mybir as mybir
from concourse.bass_utils import run_bass_kernel_spmd

P = 128
VOCAB = 100000
DIM = 512
N_POS = 4096
N_NEG = 4096
N_NEU = 2048
N_ALL = N_POS + N_NEG + N_NEU
N_CORES = 8

SLOTS = N_ALL // N_CORES           # 1280
COLS = SLOTS // P                  # 10
RCOLS = (N_POS + N_NEG) // N_CORES // P   # 8 row-gather columns (pos/neg)
# cols 8-9 are neu: element gathers land t directly in tcol

CONSISTENCY_WEIGHT = 0.5
SPARSITY_WEIGHT = 0.1
C_SP = SPARSITY_WEIGHT / (DIM - 1)
QQ = 1.0 + SPARSITY_WEIGHT         # coefficient of w inside the a-factor
PP = -(SPARSITY_WEIGHT + C_SP)     # constant inside the a-factor
RR = SPARSITY_WEIGHT               # +w*R term

# coefs tensor layout (f32, [128, CW_TOT]): ramp | dims
CW_RAMP = DIM
C_DIMS = CW_RAMP
CW_TOT = C_DIMS + RCOLS

F32 = mybir.dt.float32
I32 = mybir.dt.int32
OP = mybir.AluOpType
AF = mybir.ActivationFunctionType

_nc_cache = None


def _build_program():
    global _nc_cache
    if _nc_cache is not None:
        return _nc_cache

    nc = bacc.Bacc(
        "TRN2", target_bir_lowering=False, debug=False, num_devices=N_CORES,
        num_swdge_queues=4,
    )
    emb = nc.dram_tensor("emb", [VOCAB, DIM], F32, kind="ExternalInput")
    idx_d = nc.dram_tensor("idx32", [P, COLS], I32, kind="ExternalInput")
    coef_d = nc.dram_tensor("coefs", [P, CW_TOT], F32, kind="ExternalInput")
    out_d = nc.dram_tensor("out", [P, COLS], F32, kind="ExternalOutput")

    from contextlib import ExitStack

    with ExitStack() as ctx:
        sb = lambda name, shape, dt=F32: ctx.enter_context(
            nc.sbuf_tensor(name, shape, dt)
        )
        idx_sb = sb("idx_sb", [P, COLS], I32)
        coef_sb = sb("coef_sb", [P, CW_TOT])
        rows = sb("rows", [P, RCOLS, DIM])
        s_scr = sb("s_scr", [P, DIM])
        s_dve = sb("s_dve", [P, DIM])
        rowsum = sb("rowsum", [P, RCOLS])
        tcol = sb("tcol", [P, COLS])
        up = sb("up", [P, 4])
        wv = sb("wv", [P, COLS])
        zv = sb("zv", [P, COLS])
        av = sb("av", [P, COLS])
        xv = sb("xv", [P, RCOLS])
        L10 = sb("L10", [P, COLS])
        sem = lambda name: ctx.enter_context(nc.semaphore(name))
        io_i, io_c, io2 = sem("io_i"), sem("io_c"), sem("io2")
        gs = [sem(f"g{j}") for j in range(COLS)]
        acts, dv = sem("acts"), sem("dv")
        ramp = coef_sb[:, 0:CW_RAMP]

        # coefs (ramp+dims) only feed the DVE extraction; load via sync HWDGE.
        nc.sync.dma_start(coef_sb[:, :], coef_d[:, :]).then_inc(io_c, 16)

        blk_ctx = nc.Block()
        block = blk_ctx.__enter__()

        @block.gpsimd
        def _(gpsimd: bass.BassGpSimd):
            # Self-load the idx table: SWDGE patterned DMA, ready ~1us
            # earlier than waiting on the sync engine.
            gpsimd.dma_start(idx_sb[:, :], idx_d[:, :]).then_inc(io_i, 16)
            gpsimd.wait_ge(io_i, 16)
            # Stripe gathers across the 4 SWDGE queues so drains keep pace
            # with the CPU-bound descriptor emission.
            for j in range(RCOLS):
                inst = gpsimd.indirect_dma_start(
                    out=rows[:, j, :],
                    out_offset=None,
                    in_=emb[:, :],
                    in_offset=bass.IndirectOffsetOnAxis(
                        ap=idx_sb[:, j : j + 1], axis=0
                    ),
                ).then_inc(gs[j], 16)
                inst.ins.queue = f"qPoolDynamic{j % 4 or ''}"
            for j in range(RCOLS, COLS):
                # neu: flat element gather (idx = id*DIM+dim) lands t directly
                inst = gpsimd.indirect_dma_start(
                    out=tcol[:, j : j + 1],
                    out_offset=None,
                    in_=emb[:, :],
                    in_offset=bass.IndirectOffsetOnAxis(
                        ap=idx_sb[:, j : j + 1], axis=1
                    ),
                ).then_inc(gs[j], 16)
                inst.ins.queue = f"qPoolDynamic{j % 4 or ''}"

        @block.scalar
        def _(scalar: bass.BassEngine):
            for j in range(RCOLS):
                scalar.wait_ge(gs[j], 16)
                nc.scalar.activation(
                    s_scr[:, :], rows[:, j, :], AF.Abs,
                    accum_out=rowsum[:, j : j + 1],
                ).then_inc(acts, 1)

        @block.vector
        def _(vector: bass.BassEngine):
            n = 0

            def step(ins, wait=None, gwait=None):
                nonlocal n
                if gwait is not None:
                    vector.wait_ge(gs[gwait], 16)
                if wait is not None:
                    vector.wait_ge(dv, wait)
                ins().then_inc(dv, 1)
                n += 1
                return n

            def ext(j):
                return step(lambda: nc.vector.scalar_tensor_tensor(
                    out=s_dve[:, :],
                    in0=ramp,
                    scalar=coef_sb[:, C_DIMS + j : C_DIMS + j + 1],
                    in1=rows[:, j, :],
                    op0=OP.is_equal,
                    op1=OP.mult,
                    accum_out=tcol[:, j : j + 1],
                ), gwait=j)

            vector.wait_ge(io_c, 16)
            # Extractions per column as gathers land; the pos-half chain is
            # interleaved into the gaps. Same-engine RAW needs explicit sems
            # (deep DVE pipeline): `dv` counts completions.
            i3 = [ext(j) for j in range(4)][-1]              # 1..4
            i_up = step(lambda: nc.vector.tensor_scalar(     # u_p = -t
                out=up[:, :], in0=tcol[:, 0:4], scalar1=-1.0, scalar2=None,
                op0=OP.mult), wait=i3)
            ext(4)
            i_wp = step(lambda: nc.vector.tensor_scalar(     # w_p = (u>=0)
                out=wv[:, 0:4], in0=up[:, :], scalar1=0.0, scalar2=None,
                op0=OP.is_ge), wait=i_up)
            ext(5)
            i_zp = step(lambda: nc.vector.tensor_scalar(     # z_p = 2w-1
                out=zv[:, 0:4], in0=wv[:, 0:4], scalar1=2.0, scalar2=-1.0,
                op0=OP.mult, op1=OP.add), wait=i_wp)
            i_ap = step(lambda: nc.vector.tensor_tensor(     # a_p = u*z = |t|
                out=av[:, 0:4], in0=up[:, :], in1=zv[:, 0:4], op=OP.mult),
                wait=i_zp)
            ext(6)
            i_xp = step(lambda: nc.vector.tensor_scalar(     # x_p = Qw+Pp
                out=xv[:, 0:4], in0=wv[:, 0:4], scalar1=QQ, scalar2=PP,
                op0=OP.mult, op1=OP.add))
            i_x2 = step(lambda: nc.vector.tensor_tensor(     # x_p *= a_p
                out=xv[:, 0:4], in0=xv[:, 0:4], in1=av[:, 0:4], op=OP.mult),
                wait=max(i_ap, i_xp))
            i_yp = step(lambda: nc.vector.scalar_tensor_tensor(  # += R*w
                out=xv[:, 0:4], in0=wv[:, 0:4], scalar=RR, in1=xv[:, 0:4],
                op0=OP.mult, op1=OP.add), wait=i_x2)
            vector.wait_ge(acts, 4)
            step(lambda: nc.vector.scalar_tensor_tensor(     # L_p
                out=L10[:, 0:4], in0=rowsum[:, 0:4], scalar=C_SP,
                in1=xv[:, 0:4], op0=OP.mult, op1=OP.add), wait=i_yp)
            i7 = ext(7)
            # neg half: u = t (S=+1), so tcol is read directly.
            i_wn = step(lambda: nc.vector.tensor_scalar(
                out=wv[:, 4:8], in0=tcol[:, 4:8], scalar1=0.0, scalar2=None,
                op0=OP.is_ge), wait=i7)
            i_zn = step(lambda: nc.vector.tensor_scalar(
                out=zv[:, 4:8], in0=wv[:, 4:8], scalar1=2.0, scalar2=-1.0,
                op0=OP.mult, op1=OP.add), wait=i_wn)
            i_an = step(lambda: nc.vector.tensor_tensor(
                out=av[:, 4:8], in0=tcol[:, 4:8], in1=zv[:, 4:8], op=OP.mult),
                wait=i_zn)
            i_xn = step(lambda: nc.vector.tensor_scalar(
                out=xv[:, 4:8], in0=wv[:, 4:8], scalar1=QQ, scalar2=PP,
                op0=OP.mult, op1=OP.add))
            i_n2 = step(lambda: nc.vector.tensor_tensor(
                out=xv[:, 4:8], in0=xv[:, 4:8], in1=av[:, 4:8], op=OP.mult),
                wait=max(i_an, i_xn))
            i_yn = step(lambda: nc.vector.scalar_tensor_tensor(
                out=xv[:, 4:8], in0=wv[:, 4:8], scalar=RR, in1=xv[:, 4:8],
                op0=OP.mult, op1=OP.add), wait=i_n2)
            vector.wait_ge(acts, 8)
            step(lambda: nc.vector.scalar_tensor_tensor(
                out=L10[:, 4:8], in0=rowsum[:, 4:8], scalar=C_SP,
                in1=xv[:, 4:8], op0=OP.mult, op1=OP.add), wait=i_yn)
            # neu: L = 2|t|, t element-gathered straight into tcol[:, 8:10].
            vector.wait_ge(gs[RCOLS], 16)
            i_wu = step(lambda: nc.vector.tensor_scalar(
                out=wv[:, 8:10], in0=tcol[:, 8:10], scalar1=0.0, scalar2=None,
                op0=OP.is_ge), gwait=COLS - 1)
            i_zu = step(lambda: nc.vector.tensor_scalar(
                out=zv[:, 8:10], in0=wv[:, 8:10], scalar1=2.0, scalar2=-1.0,
                op0=OP.mult, op1=OP.add), wait=i_wu)
            i_au = step(lambda: nc.vector.tensor_tensor(
                out=av[:, 8:10], in0=tcol[:, 8:10], in1=zv[:, 8:10],
                op=OP.mult), wait=i_zu)
            step(lambda: nc.vector.tensor_scalar(
                out=L10[:, 8:10], in0=av[:, 8:10], scalar1=2.0, scalar2=None,
                op0=OP.mult), wait=i_au)
            assert n == 27, n

        @block.sync
        def _(sync: bass.BassEngine):
            sync.wait_ge(dv, 27)
            sync.dma_start(out_d[:, :], L10[:, :]).then_inc(io2, 16)
            sync.wait_ge(io2, 16)

        blk_ctx.__exit__(None, None, None)
        # The NEFF can be executed repeatedly on one load: clear our
        # semaphores after the end-of-block barrier so every run starts
        # from zero (same dance as Bass.reset()).
        ksr = nc._kernel_sem_range
        mono_start = ksr.start + 3 + (
            1 if nc._bir_kernel_barrier_sem is not None else 0
        )
        user_range = range(mono_start + len(nc._monotonic_sems), ksr.stop)
        nc.gpsimd.sem_clear(user_range)

    nc.compile()
    _nc_cache = nc
    return nc


def _deal(pos_ids, pos_dims, neg_ids, neg_dims, neu_ids, neu_dims):
    """Deal all constraints into per-core slot tables (slot j of core c =
    constraint c + 8*j of the concatenated list). Column k = slots
    [128k, 128k+128) covers global constraints [1024k, 1024(k+1)), so each
    column is single-class: cols 0-3 pos, 4-7 neg, 8-9 neu.

    Returns per-core (idx32 [128, COLS] int32, coefs [128, CW_TOT] f32).
    """
    ids = np.concatenate([pos_ids, neg_ids, neu_ids]).astype(np.int64)
    dims = np.concatenate([pos_dims, neg_dims, neu_dims]).astype(np.int64)

    idx32 = []
    coefs = []
    for c in range(N_CORES):
        g = np.arange(SLOTS) * N_CORES + c  # this core's constraints
        cid, cdim = ids[g].copy(), dims[g]
        # neu slots gather the element directly: flat index id*DIM+dim
        cid[RCOLS * P :] = cid[RCOLS * P :] * DIM + cdim[RCOLS * P :]
        # slot j -> (p = j%128, col = j//128)
        ix = np.ascontiguousarray(
            cid.reshape(COLS, P).T.astype(np.int32))  # [128, COLS]
        cf = np.zeros((P, CW_TOT), np.float32)
        cf[:, 0:CW_RAMP] = np.arange(DIM, dtype=np.float32)[None, :]
        cf[:, C_DIMS : C_DIMS + RCOLS] = cdim[: RCOLS * P].reshape(RCOLS, P).T
        idx32.append(ix)
        coefs.append(cf)
    return idx32, coefs


def _make_in_maps(emb, pos_ids, pos_dims, neg_ids, neg_dims, neu_ids, neu_dims):
    idx32, coefs = _deal(pos_ids, pos_dims, neg_ids, neg_dims, neu_ids, neu_dims)
    return [
        {"emb": emb, "idx32": idx32[c], "coefs": coefs[c]}
        for c in range(N_CORES)
    ]


def kernel(**inputs):
    emb = np.ascontiguousarray(np.asarray(inputs["embeddings"], dtype=np.float32))
    ids = {
        k: np.asarray(inputs[k]).astype(np.int64)
        for k in ("pos_ids", "pos_dims", "neg_ids", "neg_dims", "neu_ids", "neu_dims")
    }
    nc = _build_program()
    in_maps = _make_in_maps(
        emb, ids["pos_ids"], ids["pos_dims"], ids["neg_ids"], ids["neg_dims"],
        ids["neu_ids"], ids["neu_dims"],
    )
    res = run_bass_kernel_spmd(nc, in_maps, list(range(N_CORES)))
    total = sum(float(r["out"].astype(np.float64).sum()) for r in res.results)
    val = total * CONSISTENCY_WEIGHT / N_ALL
    return np.asarray(val, dtype=np.float32)


# revision 7
# speedup vs baseline: 1.5573x; 1.0036x over previous
"""DimensionalConsistencyLoss on 8 Trainium2 NeuronCores.

The loss touches only gathered rows of the [100000, 512] f32 table: 8192
pos/neg constraints read one row each (sparsity term + target element), 2048
neu constraints read one element. Everything is fetched with indirect DMA
row/element gathers (SWDGE emission is CPU-bound at ~9.3ns/descriptor, so
the 10-instruction gather window ~14.5us is the kernel's spine; everything
else is hidden under it).

Per core (1/8 of the constraints = 1280 slots = 10 columns of 128, dealt
round-robin by the host so every column is single-class: cols 0-3 pos,
4-7 neg, 8-9 neu):
  - idx table loaded by the GpSimd engine itself (SWDGE patterned DMA,
    CounterMachine emission) -- ready ~1us earlier than the HWDGE path.
  - 8x indirect-DMA row gathers (one [128,512] tile per column, striped
    over the 4 SWDGE queues), then 2x flat element gathers for neu
    (idx = id*512 + dim lands t directly); elements go LAST so the final
    instruction's drain is tiny.
  - ACT: per column, activation(Abs, accum_out) -> rowsum in one pass.
  - DVE: per column, scalar_tensor_tensor((ramp == dim) * row, accum_out)
    extracts t; the loss chain runs on [128,4] tiles with compile-time
    immediates, interleaved into the gather gaps. |t| is computed on the
    DVE as u*(2w-1) so the ACT engine is off the critical path entirely:
        pos: u = -t, neg/neu: u = t;  w = (u >= 0);  a = u*(2w-1) = |t|
        pos/neg: L = a*(1.1w - 0.1 - C_SP) + 0.1w + C_SP*rowsum
        neu:     L = 2a
  - out [128, 10] -> host sums 8 partial tiles and applies the final scale.
"""

import numpy as np

import concourse.bacc as bacc
import concourse.bass as bass
import concourse.mybir as mybir
from concourse.bass_utils import run_bass_kernel_spmd

P = 128
VOCAB = 100000
DIM = 512
N_POS = 4096
N_NEG = 4096
N_NEU = 2048
N_ALL = N_POS + N_NEG + N_NEU
N_CORES = 8

SLOTS = N_ALL // N_CORES           # 1280
COLS = SLOTS // P                  # 10
RCOLS = (N_POS + N_NEG) // N_CORES // P   # 8 row-gather columns (pos/neg)
# cols 8-9 are neu: element gathers land t directly in tcol

CONSISTENCY_WEIGHT = 0.5
SPARSITY_WEIGHT = 0.1
C_SP = SPARSITY_WEIGHT / (DIM - 1)
QQ = 1.0 + SPARSITY_WEIGHT         # coefficient of w inside the a-factor
PP = -(SPARSITY_WEIGHT + C_SP)     # constant inside the a-factor
RR = SPARSITY_WEIGHT               # +w*R term

# coefs tensor layout (f32, [128, CW_TOT]): ramp | dims
CW_RAMP = DIM
C_DIMS = CW_RAMP
CW_TOT = C_DIMS + RCOLS

F32 = mybir.dt.float32
I32 = mybir.dt.int32
OP = mybir.AluOpType
AF = mybir.ActivationFunctionType

_nc_cache = None


def _build_program():
    global _nc_cache
    if _nc_cache is not None:
        return _nc_cache

    nc = bacc.Bacc(
        "TRN2", target_bir_lowering=False, debug=False, num_devices=N_CORES,
        num_swdge_queues=4,
    )
    emb = nc.dram_tensor("emb", [VOCAB, DIM], F32, kind="ExternalInput")
    idx_d = nc.dram_tensor("idx32", [P, COLS], I32, kind="ExternalInput")
    coef_d = nc.dram_tensor("coefs", [P, CW_TOT], F32, kind="ExternalInput")
    out_d = nc.dram_tensor("out", [P, COLS], F32, kind="ExternalOutput")

    from contextlib import ExitStack

    with ExitStack() as ctx:
        sb = lambda name, shape, dt=F32: ctx.enter_context(
            nc.sbuf_tensor(name, shape, dt)
        )
        idx_sb = sb("idx_sb", [P, COLS], I32)
        coef_sb = sb("coef_sb", [P, CW_TOT])
        rows = sb("rows", [P, RCOLS, DIM])
        s_scr = sb("s_scr", [P, DIM])
        s_dve = sb("s_dve", [P, DIM])
        rowsum = sb("rowsum", [P, RCOLS])
        tcol = sb("tcol", [P, COLS])
        up = sb("up", [P, 4])
        wv = sb("wv", [P, COLS])
        zv = sb("zv", [P, COLS])
        av = sb("av", [P, COLS])
        xv = sb("xv", [P, RCOLS])
        L10 = sb("L10", [P, COLS])
        sem = lambda name: ctx.enter_context(nc.semaphore(name))
        io_i, io_c, io2 = sem("io_i"), sem("io_c"), sem("io2")
        gs = [sem(f"g{j}") for j in range(COLS)]
        acts, dv = sem("acts"), sem("dv")
        ramp = coef_sb[:, 0:CW_RAMP]

        # Issue input loads before the Block so they overlap its entry;
        # idx first (it gates the gathers), coefs second.
        nc.sync.dma_start(idx_sb[:, :], idx_d[:, :]).then_inc(io_i, 16)
        nc.sync.dma_start(coef_sb[:, :], coef_d[:, :]).then_inc(io_c, 16)

        blk_ctx = nc.Block()
        block = blk_ctx.__enter__()

        @block.gpsimd
        def _(gpsimd: bass.BassGpSimd):
            gpsimd.wait_ge(io_i, 16)
            # Stripe gathers across the 4 SWDGE queues so drains keep pace
            # with the CPU-bound descriptor emission.
            for j in range(RCOLS):
                inst = gpsimd.indirect_dma_start(
                    out=rows[:, j, :],
                    out_offset=None,
                    in_=emb[:, :],
                    in_offset=bass.IndirectOffsetOnAxis(
                        ap=idx_sb[:, j : j + 1], axis=0
                    ),
                ).then_inc(gs[j], 16)
                inst.ins.queue = f"qPoolDynamic{j % 4 or ''}"
            for j in range(RCOLS, COLS):
                # neu: flat element gather (idx = id*DIM+dim) lands t directly
                inst = gpsimd.indirect_dma_start(
                    out=tcol[:, j : j + 1],
                    out_offset=None,
                    in_=emb[:, :],
                    in_offset=bass.IndirectOffsetOnAxis(
                        ap=idx_sb[:, j : j + 1], axis=1
                    ),
                ).then_inc(gs[j], 16)
                inst.ins.queue = f"qPoolDynamic{j % 4 or ''}"

        @block.scalar
        def _(scalar: bass.BassEngine):
            for j in range(RCOLS):
                scalar.wait_ge(gs[j], 16)
                nc.scalar.activation(
                    s_scr[:, :], rows[:, j, :], AF.Abs,
                    accum_out=rowsum[:, j : j + 1],
                ).then_inc(acts, 1)

        @block.vector
        def _(vector: bass.BassEngine):
            n = 0

            def step(ins, wait=None, gwait=None):
                nonlocal n
                if gwait is not None:
                    vector.wait_ge(gs[gwait], 16)
                if wait is not None:
                    vector.wait_ge(dv, wait)
                ins().then_inc(dv, 1)
                n += 1
                return n

            def ext(j):
                return step(lambda: nc.vector.scalar_tensor_tensor(
                    out=s_dve[:, :],
                    in0=ramp,
                    scalar=coef_sb[:, C_DIMS + j : C_DIMS + j + 1],
                    in1=rows[:, j, :],
                    op0=OP.is_equal,
                    op1=OP.mult,
                    accum_out=tcol[:, j : j + 1],
                ), gwait=j)

            vector.wait_ge(io_c, 16)
            # Extractions per column as gathers land; the pos-half chain is
            # interleaved into the gaps. Same-engine RAW needs explicit sems
            # (deep DVE pipeline): `dv` counts completions.
            i3 = [ext(j) for j in range(4)][-1]              # 1..4
            i_up = step(lambda: nc.vector.tensor_scalar(     # u_p = -t
                out=up[:, :], in0=tcol[:, 0:4], scalar1=-1.0, scalar2=None,
                op0=OP.mult), wait=i3)
            ext(4)
            i_wp = step(lambda: nc.vector.tensor_scalar(     # w_p = (u>=0)
                out=wv[:, 0:4], in0=up[:, :], scalar1=0.0, scalar2=None,
                op0=OP.is_ge), wait=i_up)
            ext(5)
            i_zp = step(lambda: nc.vector.tensor_scalar(     # z_p = 2w-1
                out=zv[:, 0:4], in0=wv[:, 0:4], scalar1=2.0, scalar2=-1.0,
                op0=OP.mult, op1=OP.add), wait=i_wp)
            i_ap = step(lambda: nc.vector.tensor_tensor(     # a_p = u*z = |t|
                out=av[:, 0:4], in0=up[:, :], in1=zv[:, 0:4], op=OP.mult),
                wait=i_zp)
            ext(6)
            i_xp = step(lambda: nc.vector.tensor_scalar(     # x_p = Qw+Pp
                out=xv[:, 0:4], in0=wv[:, 0:4], scalar1=QQ, scalar2=PP,
                op0=OP.mult, op1=OP.add))
            i_x2 = step(lambda: nc.vector.tensor_tensor(     # x_p *= a_p
                out=xv[:, 0:4], in0=xv[:, 0:4], in1=av[:, 0:4], op=OP.mult),
                wait=max(i_ap, i_xp))
            i_yp = step(lambda: nc.vector.scalar_tensor_tensor(  # += R*w
                out=xv[:, 0:4], in0=wv[:, 0:4], scalar=RR, in1=xv[:, 0:4],
                op0=OP.mult, op1=OP.add), wait=i_x2)
            vector.wait_ge(acts, 4)
            step(lambda: nc.vector.scalar_tensor_tensor(     # L_p
                out=L10[:, 0:4], in0=rowsum[:, 0:4], scalar=C_SP,
                in1=xv[:, 0:4], op0=OP.mult, op1=OP.add), wait=i_yp)
            i7 = ext(7)
            # Tail: neg and neu share S=+1, so w/z/a run merged over cols
            # 4:10 (tcol read directly); then the pos/neg-only x/y/L ops on
            # cols 4:8 plus the tiny neu L. Minimizes ops after the last
            # extraction.
            vector.wait_ge(gs[RCOLS], 16)
            vector.wait_ge(gs[COLS - 1], 16)
            i_wn = step(lambda: nc.vector.tensor_scalar(
                out=wv[:, 4:10], in0=tcol[:, 4:10], scalar1=0.0, scalar2=None,
                op0=OP.is_ge), wait=i7)
            i_zn = step(lambda: nc.vector.tensor_scalar(
                out=zv[:, 4:10], in0=wv[:, 4:10], scalar1=2.0, scalar2=-1.0,
                op0=OP.mult, op1=OP.add), wait=i_wn)
            i_an = step(lambda: nc.vector.tensor_tensor(
                out=av[:, 4:10], in0=tcol[:, 4:10], in1=zv[:, 4:10],
                op=OP.mult), wait=i_zn)
            i_xn = step(lambda: nc.vector.tensor_scalar(
                out=xv[:, 4:8], in0=wv[:, 4:8], scalar1=QQ, scalar2=PP,
                op0=OP.mult, op1=OP.add))
            i_n2 = step(lambda: nc.vector.tensor_tensor(
                out=xv[:, 4:8], in0=xv[:, 4:8], in1=av[:, 4:8], op=OP.mult),
                wait=max(i_an, i_xn))
            i_yn = step(lambda: nc.vector.scalar_tensor_tensor(
                out=xv[:, 4:8], in0=wv[:, 4:8], scalar=RR, in1=xv[:, 4:8],
                op0=OP.mult, op1=OP.add), wait=i_n2)
            vector.wait_ge(acts, 8)
            step(lambda: nc.vector.scalar_tensor_tensor(
                out=L10[:, 4:8], in0=rowsum[:, 4:8], scalar=C_SP,
                in1=xv[:, 4:8], op0=OP.mult, op1=OP.add), wait=i_yn)
            step(lambda: nc.vector.tensor_scalar(
                out=L10[:, 8:10], in0=av[:, 8:10], scalar1=2.0, scalar2=None,
                op0=OP.mult))
            assert n == 24, n

        @block.sync
        def _(sync: bass.BassEngine):
            sync.wait_ge(dv, 24)
            sync.dma_start(out_d[:, :], L10[:, :]).then_inc(io2, 16)
            sync.wait_ge(io2, 16)

        blk_ctx.__exit__(None, None, None)
        # The NEFF can be executed repeatedly on one load: clear our
        # semaphores after the end-of-block barrier so every run starts
        # from zero (same dance as Bass.reset()).
        ksr = nc._kernel_sem_range
        mono_start = ksr.start + 3 + (
            1 if nc._bir_kernel_barrier_sem is not None else 0
        )
        user_range = range(mono_start + len(nc._monotonic_sems), ksr.stop)
        nc.gpsimd.sem_clear(user_range)

    nc.compile()
    _nc_cache = nc
    return nc


def _deal(pos_ids, pos_dims, neg_ids, neg_dims, neu_ids, neu_dims):
    """Deal all constraints into per-core slot tables (slot j of core c =
    constraint c + 8*j of the concatenated list). Column k = slots
    [128k, 128k+128) covers global constraints [1024k, 1024(k+1)), so each
    column is single-class: cols 0-3 pos, 4-7 neg, 8-9 neu.

    Returns per-core (idx32 [128, COLS] int32, coefs [128, CW_TOT] f32).
    """
    ids = np.concatenate([pos_ids, neg_ids, neu_ids]).astype(np.int64)
    dims = np.concatenate([pos_dims, neg_dims, neu_dims]).astype(np.int64)

    idx32 = []
    coefs = []
    for c in range(N_CORES):
        g = np.arange(SLOTS) * N_CORES + c  # this core's constraints
        cid, cdim = ids[g].copy(), dims[g]
        # neu slots gather the element directly: flat index id*DIM+dim
        cid[RCOLS * P :] = cid[RCOLS * P :] * DIM + cdim[RCOLS * P :]
        # slot j -> (p = j%128, col = j//128)
        ix = np.ascontiguousarray(
            cid.reshape(COLS, P).T.astype(np.int32))  # [128, COLS]
        cf = np.zeros((P, CW_TOT), np.float32)
        cf[:, 0:CW_RAMP] = np.arange(DIM, dtype=np.float32)[None, :]
        cf[:, C_DIMS : C_DIMS + RCOLS] = cdim[: RCOLS * P].reshape(RCOLS, P).T
        idx32.append(ix)
        coefs.append(cf)
    return idx32, coefs


def _make_in_maps(emb, pos_ids, pos_dims, neg_ids, neg_dims, neu_ids, neu_dims):
    idx32, coefs = _deal(pos_ids, pos_dims, neg_ids, neg_dims, neu_ids, neu_dims)
    return [
        {"emb": emb, "idx32": idx32[c], "coefs": coefs[c]}
        for c in range(N_CORES)
    ]


def kernel(**inputs):
    emb = np.ascontiguousarray(np.asarray(inputs["embeddings"], dtype=np.float32))
    ids = {
        k: np.asarray(inputs[k]).astype(np.int64)
        for k in ("pos_ids", "pos_dims", "neg_ids", "neg_dims", "neu_ids", "neu_dims")
    }
    nc = _build_program()
    in_maps = _make_in_maps(
        emb, ids["pos_ids"], ids["pos_dims"], ids["neg_ids"], ids["neg_dims"],
        ids["neu_ids"], ids["neu_dims"],
    )
    res = run_bass_kernel_spmd(nc, in_maps, list(range(N_CORES)))
    total = sum(float(r["out"].astype(np.float64).sum()) for r in res.results)
    val = total * CONSISTENCY_WEIGHT / N_ALL
    return np.asarray(val, dtype=np.float32)
